# revision 31
# baseline (speedup 1.0000x reference)
"""Multi-head differential attention on 8 Trainium2 NeuronCores.

Sharding: data-parallel over batch (B=2) x tensor-parallel over heads
(16 heads -> 4 per core). Core c handles batch c//4 and heads
4*(c%4) .. 4*(c%4)+3. Each core computes its heads' attention output and a
partial output projection; the host sums the 4 partials per batch.

v2: all matmuls in bf16 (2x PE rate vs fp32r: no serialized fp32
LDWEIGHTS, 1 col/cycle at 2.4 GHz). Q^T stored per-head ([128,S]: map0
dims in partitions 0:64, map1 in 64:128); K^T stored zero-padded per map
(K0[h]: map0 dims + zero half, K1[h]: zero half + map1 dims) so score
matmuls contract over full 128 partitions (64-partition matmuls run at
half rate). Output partials returned in bf16.
"""

import math
import os
import sys

sys.path.insert(0, "/opt/trn_rl_repo")

import numpy as np

B, S, HID, NH = 2, 2048, 2048, 16
HD = HID // NH          # 128
QKD = HD // 2           # 64
NCORES = 8
GRPS = NCORES // B      # head groups per batch
HPC = NH // GRPS        # heads per core = 4
LAYER_ID = 1
LAMBDA_INIT = 0.8 - 0.6 * math.exp(-0.3 * LAYER_ID)
EPS = 1e-6

NB = S // 512           # 4 seq blocks of 512
NKC = S // 128          # 16 key chunks of 128

_PROGRAM = None         # compiled bass program, reused across calls


def _build_program():
    import concourse.bass as bass
    import concourse.tile as tile
    from concourse import bacc, mybir

    f32 = mybir.dt.float32
    bf16 = mybir.dt.bfloat16
    Alu = mybir.AluOpType
    Act = mybir.ActivationFunctionType

    nc = bacc.Bacc(None, target_bir_lowering=False, debug=False)

    def din(name, shape, dt=f32):
        return nc.dram_tensor(name, shape, dt, kind="ExternalInput").ap()

    io = {
        # x: [n, p, kc, c] so one contiguous DMA covers a whole seq block
        "xq_t": din("xq_t", [NB, 128, NKC, 512], bf16),
        "xk_t": din("xk_t", [NB, 128, NKC, 512], bf16),
        "xv_t": din("xv_t", [NB, 128, NKC, 512], bf16),
        # w: [p, kc, c] — one DMA per projection
        "wq_t": din("wq_t", [128, NKC, 512], bf16),
        "wk_t": din("wk_t", [128, NKC, 512], bf16),
        "wv_t": din("wv_t", [128, NKC, 512], bf16),
        "wo_t": din("wo_t", [HPC, 128, S], bf16),
        "crep": din("crep", [128, S]),
        "srep": din("srep", [128, S]),
        "pmat": din("pmat", [128, 128], bf16),
        "tri2": din("tri2", [128, 2, 128], bf16),
        "ones_a": din("ones_a", [128, 128], bf16),
        "neglam": din("neglam", [128, 1]),
    }
    # y: [qc, p, oc, c] — one DMA per q block
    y_t = nc.dram_tensor("y_t", [NB, 128, NKC, 512], bf16,
                         kind="ExternalOutput").ap()

    from contextlib import ExitStack

    with tile.TileContext(nc) as tc, ExitStack() as ctx:
        persist = ctx.enter_context(tc.tile_pool(name="persist", bufs=1))
        constp = ctx.enter_context(tc.tile_pool(name="constp", bufs=1))

        # persistent tensors
        QT = [persist.tile([128, S], bf16, name=f"qt{h}", tag=f"qt{h}")
              for h in range(HPC)]
        K0 = [persist.tile([128, S], bf16, name=f"k0{h}", tag=f"k0{h}")
              for h in range(HPC)]
        K1 = [persist.tile([128, S], bf16, name=f"k1{h}", tag=f"k1{h}")
              for h in range(HPC)]
        VH = [persist.tile([128, 512], bf16, name=f"vh{s}", tag=f"vh{s}")
              for s in range(NKC)]
        U = [persist.tile([128, S], bf16, name=f"u{h}", tag=f"u{h}")
             for h in range(HPC)]

        # constants
        crep = constp.tile([128, S], f32, name="crep_sb", tag="crep")
        srep = constp.tile([128, S], f32, name="srep_sb", tag="srep")
        pmat = constp.tile([128, 128], bf16, name="pmat_sb", tag="pmat")
        tri2 = constp.tile([128, 2, 128], bf16, name="tri2_sb", tag="tri2")
        ones_a = constp.tile([128, 128], bf16, name="ones_a_sb", tag="ones_a")
        neglam = constp.tile([128, 1], f32, name="neglam_sb", tag="neglam")
        epsb = constp.tile([128, 1], f32, name="epsb", tag="epsb")
        nc.vector.memset(epsb[:], EPS)
        # zero halves of the padded K tiles (never written afterwards)
        for h in range(HPC):
            nc.vector.memset(K0[h][64:128, :], 0.0)
            nc.vector.memset(K1[h][0:64, :], 0.0)

        wo = [persist.tile([128, S], bf16, name=f"wo{h}", tag=f"wo{h}")
              for h in range(HPC)]

        # ------------- phase P: q/k/v projections + rope -------------
        with tc.tile_pool(name="wp", bufs=2) as wp, \
             tc.tile_pool(name="xp", bufs=2) as xp, \
             tc.tile_pool(name="pp", bufs=1, space="PSUM") as pp, \
             tc.tile_pool(name="rp", bufs=2, space="PSUM") as rp, \
             tc.tile_pool(name="rt", bufs=2) as rtp:

            def rope_sl(sl, n, name):
                px = rp.tile([128, 512], f32, name=f"px_{name}_{n}", tag="px")
                nc.tensor.matmul(px[:], pmat[:], sl, start=True, stop=True)
                tmp = rtp.tile([128, 512], bf16, name=f"rtmp_{name}_{n}",
                               tag="rtmp")
                nc.vector.tensor_mul(tmp[:], px[:],
                                     srep[:, n * 512:(n + 1) * 512])
                nc.vector.tensor_mul(sl, sl, crep[:, n * 512:(n + 1) * 512])
                nc.vector.tensor_add(sl, sl, tmp[:])

            def rope(T, n):
                rope_sl(T[:, n * 512:(n + 1) * 512], n, T.name)

            for wname, xname, mode in (("wq_t", "xq_t", 0), ("wk_t", "xk_t", 1),
                                       ("wv_t", "xv_t", 2)):
                # split DMAs: first quarter halved so compute starts early
                w0a = wp.tile([128, 2, 512], bf16, name=f"{wname}_sb0a",
                              tag="w0a")
                w0b = wp.tile([128, 2, 512], bf16, name=f"{wname}_sb0b",
                              tag="w0b")
                nc.scalar.dma_start(out=w0a[:], in_=io[wname][:, 0:2, :])
                nc.scalar.dma_start(out=w0b[:], in_=io[wname][:, 2:4, :])
                wt = [None] + [wp.tile([128, 4, 512], bf16,
                                       name=f"{wname}_sb{g}", tag=f"w{g}")
                               for g in range(1, 4)]
                for g in range(1, 4):
                    nc.scalar.dma_start(out=wt[g][:],
                                        in_=io[wname][:, 4 * g:4 * g + 4, :])

                def wloc(kc):
                    if kc < 2:
                        return w0a, kc
                    if kc < 4:
                        return w0b, kc - 2
                    return wt[kc // 4], kc % 4
                if mode == 0:
                    # consts (needed from rope onwards) go behind the first
                    # weight quarters on the scalar queue, off the x stream
                    for t, key in ((crep, "crep"), (srep, "srep"),
                                   (pmat, "pmat"), (tri2, "tri2"),
                                   (ones_a, "ones_a"), (neglam, "neglam")):
                        nc.scalar.dma_start(out=t[:], in_=io[key][:])
                xin = io[xname]
                for n in range(NB):
                    x0a = xp.tile([128, 2, 512], bf16, name=f"x_{mode}_{n}_0a",
                                  tag="x0a")
                    x0b = xp.tile([128, 2, 512], bf16, name=f"x_{mode}_{n}_0b",
                                  tag="x0b")
                    nc.sync.dma_start(out=x0a[:], in_=xin[n][:, 0:2, :])
                    nc.sync.dma_start(out=x0b[:], in_=xin[n][:, 2:4, :])
                    xck = [None] + [xp.tile([128, 4, 512], bf16,
                                            name=f"x_{mode}_{n}_{g}",
                                            tag=f"x{g}")
                                    for g in range(1, 4)]
                    for g in range(1, 4):
                        nc.sync.dma_start(out=xck[g][:],
                                          in_=xin[n][:, 4 * g:4 * g + 4, :])

                    def xloc(kc):
                        if kc < 2:
                            return x0a, kc
                        if kc < 4:
                            return x0b, kc - 2
                        return xck[kc // 4], kc % 4

                    ps = [pp.tile([128, 512], f32, name=f"pp{t}_{mode}_{n}",
                                  tag=f"pp{t}", bufs=2 if t < 2 else 1)
                          for t in range(4)]
                    for kc in range(NKC):
                        wti, wkg = wloc(kc)
                        xti, xkg = xloc(kc)
                        for t in range(4):
                            if mode < 2:
                                lhsT = wti[:, wkg, t * 128:(t + 1) * 128]
                                rhs = xti[:, xkg, :]
                            else:
                                lhsT = xti[:, xkg, t * 128:(t + 1) * 128]
                                rhs = wti[:, wkg, :]
                            nc.tensor.matmul(ps[t][:], lhsT, rhs,
                                             start=(kc == 0), stop=(kc == NKC - 1))
                    cols = slice(n * 512, (n + 1) * 512)
                    for t in range(4):
                        if mode == 0:
                            nc.vector.tensor_copy(QT[t][:, cols], ps[t][:])
                        elif mode == 1:
                            ks = rtp.tile([128, 512], bf16,
                                          name=f"ks_{t}_{n}", tag="ks",
                                          bufs=2)
                            nc.vector.tensor_copy(ks[:], ps[t][:])
                            rope_sl(ks[:], n, f"ks{t}")
                            nc.scalar.copy(K0[t][0:64, cols], ks[0:64, :])
                            nc.scalar.copy(K1[t][64:128, cols],
                                           ks[64:128, :])
                        else:
                            nc.vector.tensor_copy(VH[n * 4 + t][:], ps[t][:])
                    if mode == 2:
                        # overlap QT rope (DVE) with remaining v-proj (PE)
                        for nn in range(NB):
                            rope(QT[n], nn)
                # wo is needed only in phase Y; queue its DMA behind x
                if mode == 2:
                    for h in range(HPC):
                        nc.sync.dma_start(out=wo[h][:], in_=io["wo_t"][h])
        # ---------------- phase A: attention ----------------
        # scores^T per 128-key chunk; psum tile [128, 1024] = both maps
        with tc.tile_pool(name="sp", bufs=2, space="PSUM") as sp, \
             tc.tile_pool(name="pvp", bufs=1, space="PSUM") as pvp, \
             tc.tile_pool(name="smp", bufs=1, space="PSUM") as smp, \
             tc.tile_pool(name="ep", bufs=4) as ep, \
             tc.tile_pool(name="cb", bufs=2) as cb:
            for h in range(HPC):
                KP = (K0[h], K1[h])
                for qb in range(NB):
                    pv1 = pvp.tile([128, 512], f32, name=f"pv1_{h}_{qb}", tag="pv1")
                    pv2 = pvp.tile([128, 512], f32, name=f"pv2_{h}_{qb}", tag="pv2")
                    sm1 = smp.tile([128, 512], f32, name=f"sm1_{h}_{qb}",
                                   tag="sm1")
                    sm2 = smp.tile([128, 512], f32, name=f"sm2_{h}_{qb}",
                                   tag="sm2")
                    nkc = 4 * qb + 4

                    def scores(kc):
                        """scores + mask + exp for key chunk kc; returns
                        (E tile, qoff)."""
                        j = kc - 4 * qb  # >= 0 on the causal diagonal band
                        qoff = j * 128 if j >= 0 else 0
                        ps = sp.tile([128, 2, 512], f32,
                                     name=f"s_{h}_{qb}_{kc}", tag="s")
                        for g in (0, 1):
                            nc.tensor.matmul(
                                ps[:, g, qoff:512],
                                KP[g][:, kc * 128:(kc + 1) * 128],
                                QT[h][:, qb * 512 + qoff:(qb + 1) * 512],
                                start=True, stop=True)
                        E = ep.tile([128, 2, 512], bf16,
                                    name=f"e_{h}_{qb}_{kc}", tag="e")
                        nc.scalar.activation(E[:, :, qoff:512],
                                             ps[:, :, qoff:512],
                                             Act.Exp, scale=0.125)
                        if j >= 0:
                            # zero masked (key > query) entries of the
                            # diagonal 128x128 block, both maps at once
                            nc.vector.tensor_mul(E[:, :, qoff:qoff + 128],
                                                 E[:, :, qoff:qoff + 128],
                                                 tri2[:])
                        return E, qoff

                    # scores/exp run two chunks ahead of pv/sm so the first
                    # pv matmul is not gated on the previous block's combine
                    pend = [scores(0), scores(1)]
                    for kc in range(nkc):
                        E, qoff = pend.pop(0)
                        if kc + 2 < nkc:
                            pend.append(scores(kc + 2))
                        first, last = (kc == 0), (kc == nkc - 1)
                        for g in (0, 1):
                            pv = pv1 if g == 0 else pv2
                            nc.tensor.matmul(
                                pv[:, qoff:512],
                                VH[kc][:, h * 128:(h + 1) * 128],
                                E[:, g, qoff:512],
                                start=first, stop=last)
                            nc.tensor.matmul(
                                (sm1 if g == 0 else sm2)[:, qoff:512],
                                ones_a[:],
                                E[:, g, qoff:512],
                                start=first, stop=last)
                    # combine: U = pv1/sm1 - lam * pv2/sm2
                    # (split recip so pv1/sm psum tiles free up asap)
                    rb = cb.tile([128, 1024], f32, name=f"rb_{h}_{qb}", tag="rb")
                    t1 = cb.tile([128, 512], f32, name=f"t1_{h}_{qb}", tag="t1")
                    t2 = cb.tile([128, 512], f32, name=f"t2_{h}_{qb}", tag="t2")
                    nc.vector.reciprocal_approx_fast(rb[:, 0:512], sm1[:])
                    nc.vector.tensor_mul(t1[:], pv1[:], rb[:, 0:512])
                    nc.vector.reciprocal_approx_fast(rb[:, 512:1024], sm2[:])
                    nc.vector.tensor_mul(t2[:], pv2[:], rb[:, 512:1024])
                    # U = (t2 * -lam) + t1
                    nc.vector.scalar_tensor_tensor(
                        U[h][:, qb * 512:(qb + 1) * 512], t2[:], neglam[:], t1[:],
                        op0=Alu.mult, op1=Alu.add)

            # ---- phase N+Y: RMS norm then output projection, per q block ----
            # same pool scope as attention: no drain barrier, psum tags
            # reused (ssq -> "s", py -> alternating "pv1"/"pv2")
            nsb = cb
            ys_pool = ep

            def nblock(qc):
                for h in range(HPC):
                    usl = U[h][:, qc * 512:(qc + 1) * 512]
                    sq = nsb.tile([128, 512], bf16, name=f"sq_{h}_{qc}",
                                  tag="sq", bufs=2)
                    nc.vector.tensor_mul(sq[:], usl, usl)
                    ssq = sp.tile([128, 512], f32, name=f"ssq_{h}_{qc}", tag="s")
                    nc.tensor.matmul(ssq[:], ones_a[:], sq[:],
                                     start=True, stop=True)
                    sd = nsb.tile([128, 512], f32, name=f"sd_{h}_{qc}",
                                  tag="sd", bufs=1)
                    nc.scalar.activation(sd[:], ssq[:], Act.Sqrt,
                                         scale=1.0 / HD, bias=epsb[:])
                    rstd = nsb.tile([128, 512], f32, name=f"rstd_{h}_{qc}",
                                    tag="rstd", bufs=1)
                    nc.vector.reciprocal_approx_fast(rstd[:], sd[:])
                    nc.vector.tensor_mul(usl, usl, rstd[:])

            def yblock(qc):
                yst = ys_pool.tile([128, NKC, 512], bf16, name=f"yst_{qc}",
                                   tag="yst", bufs=2)
                for oc in range(NKC):
                    pool, tag = ((pvp, "pv1"), (pvp, "pv2"),
                                 (smp, "sm1"))[oc % 3]
                    py = pool.tile([128, 512], f32, name=f"py_{oc}_{qc}",
                                   tag=tag)
                    for h in range(HPC):
                        nc.tensor.matmul(
                            py[:],
                            wo[h][:, oc * 128:(oc + 1) * 128],
                            U[h][:, qc * 512:(qc + 1) * 512],
                            start=(h == 0), stop=(h == HPC - 1))
                    if (oc + qc) % 2 == 0:
                        nc.vector.tensor_copy(yst[:, oc, :], py[:])
                    else:
                        nc.scalar.copy(yst[:, oc, :], py[:])
                    if oc % 4 == 3:   # stream the output out in quarters
                        nc.sync.dma_start(
                            out=y_t[qc][:, oc - 3:oc + 1, :],
                            in_=yst[:, oc - 3:oc + 1, :])

            # software pipeline: N(qc+1) norm chains run on DVE/Act while
            # the PE does Y(qc) matmuls
            nblock(0)
            for qc in range(NB):
                if qc + 1 < NB:
                    nblock(qc + 1)
                yblock(qc)

    nc.compile()
    return nc


def _host_prep(q, k, v, Wq, Wk, Wv, Wo, lambda_q1, lambda_k1, lambda_q2,
               lambda_k2, gnorm_w, cos_emb, sin_emb):
    import ml_dtypes

    f32 = np.float32
    bf = ml_dtypes.bfloat16
    q = np.asarray(q, f32); k = np.asarray(k, f32); v = np.asarray(v, f32)
    Wq = np.asarray(Wq, f32); Wk = np.asarray(Wk, f32)
    Wv = np.asarray(Wv, f32); Wo = np.asarray(Wo, f32)
    gnorm_w = np.asarray(gnorm_w, f32)
    cos_emb = np.asarray(cos_emb, f32); sin_emb = np.asarray(sin_emb, f32)

    lam1 = np.exp(np.sum(np.asarray(lambda_q1, f32) * np.asarray(lambda_k1, f32),
                         dtype=f32))
    lam2 = np.exp(np.sum(np.asarray(lambda_q2, f32) * np.asarray(lambda_k2, f32),
                         dtype=f32))
    lam = np.float32(lam1 - lam2 + LAMBDA_INIT)

    # per-batch activations in [n, p, kc, c] DMA layout (bf16):
    #   x_prep[n, p, kc, c] = x[b][n*512 + c, kc*128 + p]
    def xprep(xb):
        a = xb.T.reshape(NKC, 128, NB, 512)        # [kc, p, n, c]
        return np.ascontiguousarray(a.transpose(2, 1, 0, 3).astype(bf))

    xt = {}
    for b in range(B):
        xt[("q", b)] = xprep(q[b])
        xt[("k", b)] = xprep(k[b])
        xt[("v", b)] = xprep(v[b])

    # shared constant tensors
    base_c = cos_emb[:S, :QKD]          # [S, 64]
    base_s = sin_emb[:S, :QKD]
    crep = np.ascontiguousarray(np.tile(base_c.T, (2, 1)))   # [128, S] f32
    srep = np.ascontiguousarray(np.tile(base_s.T, (2, 1)))
    pmat = np.zeros((128, 128), f32)
    for blk in range(2):
        o = blk * 64
        for i in range(QKD // 2):
            pmat[o + 2 * i, o + 2 * i + 1] = 1.0     # lhsT[2i, 2i+1]
            pmat[o + 2 * i + 1, o + 2 * i] = -1.0    # lhsT[2i+1, 2i]
    pmat = pmat.astype(bf)
    # tri2[p, g, c] = 0 if p > c (key index > query index) else 1
    tri01 = np.triu(np.ones((128, 128), f32))
    tri2 = np.ascontiguousarray(
        np.broadcast_to(tri01[:, None, :], (128, 2, 128)).astype(bf))
    ones_a = np.ones((128, 128), bf)
    neglam = np.full((128, 1), -lam, f32)

    per_core = []
    for c in range(NCORES):
        b, grp = c // GRPS, c % GRPS
        # per-head contiguous feature layout: tile h = head grp*4+h,
        # partitions 0:64 = map0 dims, 64:128 = map1 dims
        rows = slice(grp * 512, (grp + 1) * 512)

        def wprep(W):
            # [p, kc, c] layout: w_prep[p, kc, c] = W[rows][c, kc*128 + p]
            a = W[rows, :].T.reshape(NKC, 128, 512)   # [kc, p, c]
            return np.ascontiguousarray(a.transpose(1, 0, 2).astype(bf))

        wq_t = wprep(Wq)
        wk_t = wprep(Wk)
        wv_t = wprep(Wv)
        gtile = np.tile(gnorm_w, HPC)                       # [512]
        # wo_t[h, p, c] = scaled Wo[c, grp*512 + h*128 + p]
        wo_full = ((1.0 - LAMBDA_INIT) * Wo[:, rows] * gtile[None, :]).T
        wo_t = np.ascontiguousarray(
            wo_full.reshape(HPC, 128, HID).astype(bf))
        per_core.append({
            "xq_t": xt[("q", b)], "xk_t": xt[("k", b)], "xv_t": xt[("v", b)],
            "wq_t": wq_t, "wk_t": wk_t, "wv_t": wv_t, "wo_t": wo_t,
            "crep": crep, "srep": srep, "pmat": pmat,
            "tri2": tri2, "ones_a": ones_a, "neglam": neglam,
        })
    return per_core


def _install_ntff_hook():
    """antenv.axon_hooks is absent in this image; synthesize it so
    run_bass_kernel_spmd(trace=True) can capture NTFF profiles."""
    import sys as _sys
    import types

    if "antenv.axon_hooks" in _sys.modules:
        return
    import antenv
    mod = types.ModuleType("antenv.axon_hooks")
    state = {"hook": None}
    mod.set_axon_ntff_profile_hook = lambda h: state.__setitem__("hook", h)
    mod.get_axon_ntff_profile_hook = lambda: state["hook"]
    _sys.modules["antenv.axon_hooks"] = mod
    antenv.axon_hooks = mod
    try:
        from trn_agent_boot.trn_boot import _ntff_profile_via_ctypes
        state["hook"] = _ntff_profile_via_ctypes("/opt/axon/libaxon_pjrt.so")
    except Exception as e:  # degrade: trace skipped, run still works
        print("ntff hook install failed:", e)


def kernel(q, k, v, Wq, Wk, Wv, Wo, lambda_q1, lambda_k1, lambda_q2,
           lambda_k2, gnorm_w, cos_emb, sin_emb, mask, _trace=False):
    if _trace:
        _install_ntff_hook()
    global _PROGRAM
    if _PROGRAM is None:
        _PROGRAM = _build_program()
    nc = _PROGRAM

    in_maps = _host_prep(q, k, v, Wq, Wk, Wv, Wo, lambda_q1, lambda_k1,
                         lambda_q2, lambda_k2, gnorm_w, cos_emb, sin_emb)

    from concourse.bass_utils import run_bass_kernel_spmd
    res = run_bass_kernel_spmd(nc, in_maps, core_ids=list(range(NCORES)),
                               trace=_trace)
    kernel.last_result = res

    y = np.zeros((B, S, HID), np.float32)
    for c in range(NCORES):
        # y_t: [qc, p, oc, c] -> [HID, S] via (oc, p) rows, (qc, c) cols
        yt = res.results[c]["y_t"].astype(np.float32)
        yt = yt.transpose(2, 1, 0, 3).reshape(HID, S)
        y[c // GRPS] += yt.T
    return y


# revision 32
# speedup vs baseline: 1.1809x; 1.1809x over previous
"""Multi-head differential attention on 8 Trainium2 NeuronCores.

Sharding: data-parallel over batch (B=2) x tensor-parallel over heads
(16 heads -> 4 per core). Core c handles batch c//4 and heads
4*(c%4) .. 4*(c%4)+3. Each core computes its heads' attention output and a
partial output projection; the host sums the 4 partials per batch.

v2: all matmuls in bf16 (2x PE rate vs fp32r: no serialized fp32
LDWEIGHTS, 1 col/cycle at 2.4 GHz). Q^T stored per-head ([128,S]: map0
dims in partitions 0:64, map1 in 64:128); K^T stored zero-padded per map
(K0[h]: map0 dims + zero half, K1[h]: zero half + map1 dims) so score
matmuls contract over full 128 partitions (64-partition matmuls run at
half rate). Output partials returned in bf16.
"""

import math
import os
import sys

sys.path.insert(0, "/opt/trn_rl_repo")

import numpy as np

B, S, HID, NH = 2, 2048, 2048, 16
HD = HID // NH          # 128
QKD = HD // 2           # 64
NCORES = 8
GRPS = NCORES // B      # head groups per batch
HPC = NH // GRPS        # heads per core = 4
LAYER_ID = 1
LAMBDA_INIT = 0.8 - 0.6 * math.exp(-0.3 * LAYER_ID)
EPS = 1e-6

NB = S // 512           # 4 seq blocks of 512
NKC = S // 128          # 16 key chunks of 128

_PROGRAM = None         # compiled bass program, reused across calls


def _build_program():
    import concourse.bass as bass
    import concourse.tile as tile
    from concourse import bacc, mybir

    f32 = mybir.dt.float32
    bf16 = mybir.dt.bfloat16
    Alu = mybir.AluOpType
    Act = mybir.ActivationFunctionType

    nc = bacc.Bacc(None, target_bir_lowering=False, debug=False)

    def din(name, shape, dt=f32):
        return nc.dram_tensor(name, shape, dt, kind="ExternalInput").ap()

    io = {
        # x: [n, p, kc, c] so one contiguous DMA covers a whole seq block
        "xq_t": din("xq_t", [NB, 128, NKC, 512], bf16),
        "xk_t": din("xk_t", [NB, 128, NKC, 512], bf16),
        "xv_t": din("xv_t", [NB, 128, NKC, 512], bf16),
        # w: [p, kc, c] — one DMA per projection
        "wq_t": din("wq_t", [128, NKC, 512], bf16),
        "wk_t": din("wk_t", [128, NKC, 512], bf16),
        "wv_t": din("wv_t", [128, NKC, 512], bf16),
        "wo_t": din("wo_t", [HPC, 128, S], bf16),
        "crep": din("crep", [128, S]),
        "srep": din("srep", [128, S]),
        "pmat": din("pmat", [128, 128], bf16),
        "tri2": din("tri2", [128, 2, 128], bf16),
        "ones_a": din("ones_a", [128, 128], bf16),
        "neglam": din("neglam", [128, 1]),
    }
    # y: [qc, p, oc, c] — one DMA per q block
    y_t = nc.dram_tensor("y_t", [NB, 128, NKC, 512], bf16,
                         kind="ExternalOutput").ap()

    from contextlib import ExitStack

    with tile.TileContext(nc) as tc, ExitStack() as ctx:
        persist = ctx.enter_context(tc.tile_pool(name="persist", bufs=1))
        constp = ctx.enter_context(tc.tile_pool(name="constp", bufs=1))

        # persistent tensors
        QT = [persist.tile([128, S], bf16, name=f"qt{h}", tag=f"qt{h}")
              for h in range(HPC)]
        K0 = [persist.tile([128, S], bf16, name=f"k0{h}", tag=f"k0{h}")
              for h in range(HPC)]
        K1 = [persist.tile([128, S], bf16, name=f"k1{h}", tag=f"k1{h}")
              for h in range(HPC)]
        VH = [persist.tile([128, 512], bf16, name=f"vh{s}", tag=f"vh{s}")
              for s in range(NKC)]
        U = [persist.tile([128, S], bf16, name=f"u{h}", tag=f"u{h}")
             for h in range(HPC)]

        # constants
        crep = constp.tile([128, S], f32, name="crep_sb", tag="crep")
        srep = constp.tile([128, S], f32, name="srep_sb", tag="srep")
        pmat = constp.tile([128, 128], bf16, name="pmat_sb", tag="pmat")
        tri2 = constp.tile([128, 2, 128], bf16, name="tri2_sb", tag="tri2")
        ones_a = constp.tile([128, 128], bf16, name="ones_a_sb", tag="ones_a")
        neglam = constp.tile([128, 1], f32, name="neglam_sb", tag="neglam")
        epsb = constp.tile([128, 1], f32, name="epsb", tag="epsb")
        nc.vector.memset(epsb[:], EPS)
        # zero halves of the padded K tiles (never written afterwards)
        for h in range(HPC):
            nc.vector.memset(K0[h][64:128, :], 0.0)
            nc.vector.memset(K1[h][0:64, :], 0.0)

        wo = [persist.tile([128, S], bf16, name=f"wo{h}", tag=f"wo{h}")
              for h in range(HPC)]

        # ------------- phase P: q/k/v projections + rope -------------
        with tc.tile_pool(name="wp", bufs=2) as wp, \
             tc.tile_pool(name="xp", bufs=2) as xp, \
             tc.tile_pool(name="pp", bufs=1, space="PSUM") as pp, \
             tc.tile_pool(name="rp", bufs=2, space="PSUM") as rp, \
             tc.tile_pool(name="rt", bufs=2) as rtp:

            def rope_sl(sl, n, name):
                px = rp.tile([128, 512], f32, name=f"px_{name}_{n}", tag="px")
                nc.tensor.matmul(px[:], pmat[:], sl, start=True, stop=True)
                tmp = rtp.tile([128, 512], bf16, name=f"rtmp_{name}_{n}",
                               tag="rtmp")
                nc.vector.tensor_mul(tmp[:], px[:],
                                     srep[:, n * 512:(n + 1) * 512])
                nc.vector.tensor_mul(sl, sl, crep[:, n * 512:(n + 1) * 512])
                nc.vector.tensor_add(sl, sl, tmp[:])

            def rope(T, n):
                rope_sl(T[:, n * 512:(n + 1) * 512], n, T.name)

            for wname, xname, mode in (("wq_t", "xq_t", 0), ("wk_t", "xk_t", 1),
                                       ("wv_t", "xv_t", 2)):
                # split DMAs: first quarter halved so compute starts early
                w0a = wp.tile([128, 2, 512], bf16, name=f"{wname}_sb0a",
                              tag="w0a")
                w0b = wp.tile([128, 2, 512], bf16, name=f"{wname}_sb0b",
                              tag="w0b")
                nc.scalar.dma_start(out=w0a[:], in_=io[wname][:, 0:2, :])
                nc.scalar.dma_start(out=w0b[:], in_=io[wname][:, 2:4, :])
                wt = [None] + [wp.tile([128, 4, 512], bf16,
                                       name=f"{wname}_sb{g}", tag=f"w{g}")
                               for g in range(1, 4)]
                for g in range(1, 4):
                    nc.scalar.dma_start(out=wt[g][:],
                                        in_=io[wname][:, 4 * g:4 * g + 4, :])

                def wloc(kc):
                    if kc < 2:
                        return w0a, kc
                    if kc < 4:
                        return w0b, kc - 2
                    return wt[kc // 4], kc % 4
                if mode == 0:
                    # consts (needed from rope onwards) go behind the first
                    # weight quarters on the scalar queue, off the x stream
                    for t, key in ((crep, "crep"), (srep, "srep"),
                                   (pmat, "pmat"), (tri2, "tri2"),
                                   (ones_a, "ones_a"), (neglam, "neglam")):
                        nc.scalar.dma_start(out=t[:], in_=io[key][:])
                xin = io[xname]
                for n in range(NB):
                    x0a = xp.tile([128, 2, 512], bf16, name=f"x_{mode}_{n}_0a",
                                  tag="x0a")
                    x0b = xp.tile([128, 2, 512], bf16, name=f"x_{mode}_{n}_0b",
                                  tag="x0b")
                    nc.sync.dma_start(out=x0a[:], in_=xin[n][:, 0:2, :])
                    nc.sync.dma_start(out=x0b[:], in_=xin[n][:, 2:4, :])
                    xck = [None] + [xp.tile([128, 4, 512], bf16,
                                            name=f"x_{mode}_{n}_{g}",
                                            tag=f"x{g}")
                                    for g in range(1, 4)]
                    for g in range(1, 4):
                        nc.sync.dma_start(out=xck[g][:],
                                          in_=xin[n][:, 4 * g:4 * g + 4, :])

                    def xloc(kc):
                        if kc < 2:
                            return x0a, kc
                        if kc < 4:
                            return x0b, kc - 2
                        return xck[kc // 4], kc % 4

                    ps = [pp.tile([128, 512], f32, name=f"pp{t}_{mode}_{n}",
                                  tag=f"pp{t}") for t in range(4)]
                    for kc in range(NKC):
                        wti, wkg = wloc(kc)
                        xti, xkg = xloc(kc)
                        for t in range(4):
                            if mode < 2:
                                lhsT = wti[:, wkg, t * 128:(t + 1) * 128]
                                rhs = xti[:, xkg, :]
                            else:
                                lhsT = xti[:, xkg, t * 128:(t + 1) * 128]
                                rhs = wti[:, wkg, :]
                            nc.tensor.matmul(ps[t][:], lhsT, rhs,
                                             start=(kc == 0), stop=(kc == NKC - 1))
                    cols = slice(n * 512, (n + 1) * 512)
                    for t in range(4):
                        if mode == 0:
                            nc.vector.tensor_copy(QT[t][:, cols], ps[t][:])
                        elif mode == 1:
                            ks = rtp.tile([128, 512], bf16,
                                          name=f"ks_{t}_{n}", tag="ks",
                                          bufs=2)
                            nc.vector.tensor_copy(ks[:], ps[t][:])
                            rope_sl(ks[:], n, f"ks{t}")
                            nc.scalar.copy(K0[t][0:64, cols], ks[0:64, :])
                            nc.scalar.copy(K1[t][64:128, cols],
                                           ks[64:128, :])
                        else:
                            nc.vector.tensor_copy(VH[n * 4 + t][:], ps[t][:])
                    if mode == 2:
                        # overlap QT rope (DVE) with remaining v-proj (PE)
                        for nn in range(NB):
                            rope(QT[n], nn)
                # wo is needed only in phase Y; queue its DMA behind x
                if mode == 2:
                    for h in range(HPC):
                        nc.sync.dma_start(out=wo[h][:], in_=io["wo_t"][h])
        # ---------------- phase A: attention ----------------
        # scores^T per 128-key chunk; psum tile [128, 1024] = both maps
        with tc.tile_pool(name="sp", bufs=2, space="PSUM") as sp, \
             tc.tile_pool(name="pvp", bufs=1, space="PSUM") as pvp, \
             tc.tile_pool(name="smp", bufs=1, space="PSUM") as smp, \
             tc.tile_pool(name="ep", bufs=4) as ep, \
             tc.tile_pool(name="cb", bufs=2) as cb:
            for h in range(HPC):
                KP = (K0[h], K1[h])
                for qb in range(NB):
                    pv1 = pvp.tile([128, 512], f32, name=f"pv1_{h}_{qb}", tag="pv1")
                    pv2 = pvp.tile([128, 512], f32, name=f"pv2_{h}_{qb}", tag="pv2")
                    sm1 = smp.tile([128, 512], f32, name=f"sm1_{h}_{qb}",
                                   tag="sm1")
                    sm2 = smp.tile([128, 512], f32, name=f"sm2_{h}_{qb}",
                                   tag="sm2")
                    nkc = 4 * qb + 4

                    def scores(kc):
                        """scores + mask + exp for key chunk kc; returns
                        (E tile, qoff)."""
                        j = kc - 4 * qb  # >= 0 on the causal diagonal band
                        qoff = j * 128 if j >= 0 else 0
                        ps = sp.tile([128, 2, 512], f32,
                                     name=f"s_{h}_{qb}_{kc}", tag="s")
                        for g in (0, 1):
                            nc.tensor.matmul(
                                ps[:, g, qoff:512],
                                KP[g][:, kc * 128:(kc + 1) * 128],
                                QT[h][:, qb * 512 + qoff:(qb + 1) * 512],
                                start=True, stop=True)
                        E = ep.tile([128, 2, 512], bf16,
                                    name=f"e_{h}_{qb}_{kc}", tag="e")
                        nc.scalar.activation(E[:, :, qoff:512],
                                             ps[:, :, qoff:512],
                                             Act.Exp, scale=0.125)
                        if j >= 0:
                            # zero masked (key > query) entries of the
                            # diagonal 128x128 block, both maps at once
                            nc.vector.tensor_mul(E[:, :, qoff:qoff + 128],
                                                 E[:, :, qoff:qoff + 128],
                                                 tri2[:])
                        return E, qoff

                    # scores/exp run two chunks ahead of pv/sm so the first
                    # pv matmul is not gated on the previous block's combine
                    pend = [scores(0), scores(1)]
                    for kc in range(nkc):
                        E, qoff = pend.pop(0)
                        if kc + 2 < nkc:
                            pend.append(scores(kc + 2))
                        first, last = (kc == 0), (kc == nkc - 1)
                        for g in (0, 1):
                            pv = pv1 if g == 0 else pv2
                            nc.tensor.matmul(
                                pv[:, qoff:512],
                                VH[kc][:, h * 128:(h + 1) * 128],
                                E[:, g, qoff:512],
                                start=first, stop=last)
                            nc.tensor.matmul(
                                (sm1 if g == 0 else sm2)[:, qoff:512],
                                ones_a[:],
                                E[:, g, qoff:512],
                                start=first, stop=last)
                    # combine: U = pv1/sm1 - lam * pv2/sm2
                    # (split recip so pv1/sm psum tiles free up asap)
                    rb = cb.tile([128, 1024], f32, name=f"rb_{h}_{qb}", tag="rb")
                    t1 = cb.tile([128, 512], f32, name=f"t1_{h}_{qb}", tag="t1")
                    t2 = cb.tile([128, 512], f32, name=f"t2_{h}_{qb}", tag="t2")
                    nc.vector.reciprocal_approx_fast(rb[:, 0:512], sm1[:])
                    nc.vector.tensor_mul(t1[:], pv1[:], rb[:, 0:512])
                    nc.vector.reciprocal_approx_fast(rb[:, 512:1024], sm2[:])
                    nc.vector.tensor_mul(t2[:], pv2[:], rb[:, 512:1024])
                    # U = (t2 * -lam) + t1
                    nc.vector.scalar_tensor_tensor(
                        U[h][:, qb * 512:(qb + 1) * 512], t2[:], neglam[:], t1[:],
                        op0=Alu.mult, op1=Alu.add)

            # ---- phase N+Y: RMS norm then output projection, per q block ----
            # same pool scope as attention: no drain barrier, psum tags
            # reused (ssq -> "s", py -> alternating "pv1"/"pv2")
            nsb = cb
            ys_pool = ep

            def nblock(qc):
                for h in range(HPC):
                    usl = U[h][:, qc * 512:(qc + 1) * 512]
                    sq = nsb.tile([128, 512], bf16, name=f"sq_{h}_{qc}",
                                  tag="sq", bufs=2)
                    nc.vector.tensor_mul(sq[:], usl, usl)
                    ssq = sp.tile([128, 512], f32, name=f"ssq_{h}_{qc}", tag="s")
                    nc.tensor.matmul(ssq[:], ones_a[:], sq[:],
                                     start=True, stop=True)
                    sd = nsb.tile([128, 512], f32, name=f"sd_{h}_{qc}",
                                  tag="sd", bufs=1)
                    nc.scalar.activation(sd[:], ssq[:], Act.Sqrt,
                                         scale=1.0 / HD, bias=epsb[:])
                    rstd = nsb.tile([128, 512], f32, name=f"rstd_{h}_{qc}",
                                    tag="rstd", bufs=1)
                    nc.vector.reciprocal_approx_fast(rstd[:], sd[:])
                    nc.vector.tensor_mul(usl, usl, rstd[:])

            def yblock(qc):
                yst = ys_pool.tile([128, NKC, 512], bf16, name=f"yst_{qc}",
                                   tag="yst", bufs=2)
                for oc in range(NKC):
                    py = pvp.tile([128, 512], f32, name=f"py_{oc}_{qc}",
                                  tag="pv1" if oc % 2 == 0 else "pv2")
                    for h in range(HPC):
                        nc.tensor.matmul(
                            py[:],
                            wo[h][:, oc * 128:(oc + 1) * 128],
                            U[h][:, qc * 512:(qc + 1) * 512],
                            start=(h == 0), stop=(h == HPC - 1))
                    if (oc + qc) % 2 == 0:
                        nc.vector.tensor_copy(yst[:, oc, :], py[:])
                    else:
                        nc.scalar.copy(yst[:, oc, :], py[:])
                    if oc % 4 == 3:   # stream the output out in quarters
                        nc.sync.dma_start(
                            out=y_t[qc][:, oc - 3:oc + 1, :],
                            in_=yst[:, oc - 3:oc + 1, :])

            # software pipeline: N(qc+1) norm chains run on DVE/Act while
            # the PE does Y(qc) matmuls
            nblock(0)
            for qc in range(NB):
                if qc + 1 < NB:
                    nblock(qc + 1)
                yblock(qc)

    nc.compile()
    return nc


def _host_prep(q, k, v, Wq, Wk, Wv, Wo, lambda_q1, lambda_k1, lambda_q2,
               lambda_k2, gnorm_w, cos_emb, sin_emb):
    import ml_dtypes

    f32 = np.float32
    bf = ml_dtypes.bfloat16
    q = np.asarray(q, f32); k = np.asarray(k, f32); v = np.asarray(v, f32)
    Wq = np.asarray(Wq, f32); Wk = np.asarray(Wk, f32)
    Wv = np.asarray(Wv, f32); Wo = np.asarray(Wo, f32)
    gnorm_w = np.asarray(gnorm_w, f32)
    cos_emb = np.asarray(cos_emb, f32); sin_emb = np.asarray(sin_emb, f32)

    lam1 = np.exp(np.sum(np.asarray(lambda_q1, f32) * np.asarray(lambda_k1, f32),
                         dtype=f32))
    lam2 = np.exp(np.sum(np.asarray(lambda_q2, f32) * np.asarray(lambda_k2, f32),
                         dtype=f32))
    lam = np.float32(lam1 - lam2 + LAMBDA_INIT)

    # per-batch activations in [n, p, kc, c] DMA layout (bf16):
    #   x_prep[n, p, kc, c] = x[b][n*512 + c, kc*128 + p]
    def xprep(xb):
        a = xb.T.reshape(NKC, 128, NB, 512)        # [kc, p, n, c]
        return np.ascontiguousarray(a.transpose(2, 1, 0, 3).astype(bf))

    xt = {}
    for b in range(B):
        xt[("q", b)] = xprep(q[b])
        xt[("k", b)] = xprep(k[b])
        xt[("v", b)] = xprep(v[b])

    # shared constant tensors
    base_c = cos_emb[:S, :QKD]          # [S, 64]
    base_s = sin_emb[:S, :QKD]
    crep = np.ascontiguousarray(np.tile(base_c.T, (2, 1)))   # [128, S] f32
    srep = np.ascontiguousarray(np.tile(base_s.T, (2, 1)))
    pmat = np.zeros((128, 128), f32)
    for blk in range(2):
        o = blk * 64
        for i in range(QKD // 2):
            pmat[o + 2 * i, o + 2 * i + 1] = 1.0     # lhsT[2i, 2i+1]
            pmat[o + 2 * i + 1, o + 2 * i] = -1.0    # lhsT[2i+1, 2i]
    pmat = pmat.astype(bf)
    # tri2[p, g, c] = 0 if p > c (key index > query index) else 1
    tri01 = np.triu(np.ones((128, 128), f32))
    tri2 = np.ascontiguousarray(
        np.broadcast_to(tri01[:, None, :], (128, 2, 128)).astype(bf))
    ones_a = np.ones((128, 128), bf)
    neglam = np.full((128, 1), -lam, f32)

    per_core = []
    for c in range(NCORES):
        b, grp = c // GRPS, c % GRPS
        # per-head contiguous feature layout: tile h = head grp*4+h,
        # partitions 0:64 = map0 dims, 64:128 = map1 dims
        rows = slice(grp * 512, (grp + 1) * 512)

        def wprep(W):
            # [p, kc, c] layout: w_prep[p, kc, c] = W[rows][c, kc*128 + p]
            a = W[rows, :].T.reshape(NKC, 128, 512)   # [kc, p, c]
            return np.ascontiguousarray(a.transpose(1, 0, 2).astype(bf))

        wq_t = wprep(Wq)
        wk_t = wprep(Wk)
        wv_t = wprep(Wv)
        gtile = np.tile(gnorm_w, HPC)                       # [512]
        # wo_t[h, p, c] = scaled Wo[c, grp*512 + h*128 + p]
        wo_full = ((1.0 - LAMBDA_INIT) * Wo[:, rows] * gtile[None, :]).T
        wo_t = np.ascontiguousarray(
            wo_full.reshape(HPC, 128, HID).astype(bf))
        per_core.append({
            "xq_t": xt[("q", b)], "xk_t": xt[("k", b)], "xv_t": xt[("v", b)],
            "wq_t": wq_t, "wk_t": wk_t, "wv_t": wv_t, "wo_t": wo_t,
            "crep": crep, "srep": srep, "pmat": pmat,
            "tri2": tri2, "ones_a": ones_a, "neglam": neglam,
        })
    return per_core


def _install_ntff_hook():
    """antenv.axon_hooks is absent in this image; synthesize it so
    run_bass_kernel_spmd(trace=True) can capture NTFF profiles."""
    import sys as _sys
    import types

    if "antenv.axon_hooks" in _sys.modules:
        return
    import antenv
    mod = types.ModuleType("antenv.axon_hooks")
    state = {"hook": None}
    mod.set_axon_ntff_profile_hook = lambda h: state.__setitem__("hook", h)
    mod.get_axon_ntff_profile_hook = lambda: state["hook"]
    _sys.modules["antenv.axon_hooks"] = mod
    antenv.axon_hooks = mod
    try:
        from trn_agent_boot.trn_boot import _ntff_profile_via_ctypes
        state["hook"] = _ntff_profile_via_ctypes("/opt/axon/libaxon_pjrt.so")
    except Exception as e:  # degrade: trace skipped, run still works
        print("ntff hook install failed:", e)


def kernel(q, k, v, Wq, Wk, Wv, Wo, lambda_q1, lambda_k1, lambda_q2,
           lambda_k2, gnorm_w, cos_emb, sin_emb, mask, _trace=False):
    if _trace:
        _install_ntff_hook()
    global _PROGRAM
    if _PROGRAM is None:
        _PROGRAM = _build_program()
    nc = _PROGRAM

    in_maps = _host_prep(q, k, v, Wq, Wk, Wv, Wo, lambda_q1, lambda_k1,
                         lambda_q2, lambda_k2, gnorm_w, cos_emb, sin_emb)

    from concourse.bass_utils import run_bass_kernel_spmd
    res = run_bass_kernel_spmd(nc, in_maps, core_ids=list(range(NCORES)),
                               trace=_trace)
    kernel.last_result = res

    y = np.zeros((B, S, HID), np.float32)
    for c in range(NCORES):
        # y_t: [qc, p, oc, c] -> [HID, S] via (oc, p) rows, (qc, c) cols
        yt = res.results[c]["y_t"].astype(np.float32)
        yt = yt.transpose(2, 1, 0, 3).reshape(HID, S)
        y[c // GRPS] += yt.T
    return y
